# revision 23
# baseline (speedup 1.0000x reference)
"""
Trainium2 Bass kernel for nn_AtomBondConsistencyLayer.

Math (see reference):
  atom_logits = silu(h@Wa1+ba1)@Wa2+ba2                         [N,119]
  bond_logits = silu(h[ii]@Wb1[:D] + h[jj]@Wb1[D:] + bb1)@Wb2+bb2  [P,5]
  valency    = silu(h@Wv1[:D] + softmax(atom_logits)@Wv1[D:]+bv1)@Wv2+bv2
  atom_pairs = triu pairs (i<j)                                  [P,2]

Key algebraic identity: h[ii]@W == (h@W)[ii], so the two huge [P,256]@[256,256]
GEMMs collapse to A=h@Wb1[:D], B=h@Wb1[D:] ([N,256] each, computed on host —
0.1% of FLOPs) plus the dominant per-pair work which runs on 8 NeuronCores:

  for each row-group i (pairs (i, j) with j=i+1..N-1):
      bond_hidden.T[:, j] = silu(B.T[:, j] + (A.T[:, i] + bb1))   <- ACT engine,
          bias folded into the activation's per-partition bias operand
      bond_logits.T = Wb2.T @ bond_hidden.T                        <- PE, bf16
  (+ bb2 added on host during assembly)

Sharding: group i -> core (i mod 8), so every core gets ~P/8 pairs AND ~128
groups (balanced instruction count). SPMD requires one identical graph on all
cores, so per-core inputs are pre-shifted slabs: slab_c[:, col] = B.T[:, col+c]
and slot widths padded to the core-0 width; junk columns are discarded on host.
"""

import sys
import numpy as np

sys.path.insert(0, "/opt/trn_rl_repo")

N = 1024
D = 256
NCORES = 8
NSLOT = 128  # slots per core; slot s on core c handles group i = 8*s + c
FP = [N - 1 - 8 * s for s in range(NSLOT)]  # padded slot widths (core-0 width)
OFFS = np.concatenate([[0], np.cumsum(FP)]).astype(np.int64)
TOT = int(OFFS[-1])  # 65920 padded pairs per core
P = N * (N - 1) // 2  # 523776

_NC_CACHE = {}

# matmul chunks of <=512 free columns, in slot order
CHUNKS = []
for _s in range(NSLOT):
    _c0 = 0
    while _c0 < FP[_s]:
        CHUNKS.append((_s, _c0, min(512, FP[_s] - _c0)))
        _c0 += 512
N_OUT_CHUNKS = 16  # output DMA granularity (slot-aligned)
# slots offloaded to DVE via polynomial silu (~17% of elements), interleaved
# with ACT slots so both engines stream from the start
DVE_SLOTS = frozenset(range(0, 39, 3))
# degree-5 odd minimax-ish fit of tanh(x/2) on the data distribution:
# z = v*(PC0 + u*(PC1 + PC2*u)), u=v^2; 2*silu(v) ~= v + v*clamp(z,-1,1)
PC0, PC1, PC2 = 0.499859, -0.04079, 0.003025


def _register_dve_ops():
    import concourse.dve_ops as dve_ops
    from concourse.dve_spec import (
        Spec, Src0, Src1, C0, C1, C2, C3, Zero, One,
        sq, maxx, minn, lower, _spill_c3_to_src1,
    )
    from concourse.dve_uop import DveOpSpec

    if "SILU_Z_ANT" in dve_ops._SUB_OPCODE_FOR_NAME:
        return (dve_ops._BY_NAME["SILU_Z_ANT"], dve_ops._BY_NAME["SILU_FIN_ANT"])

    def _np_silu_z(in0, in1, s0, s1, imm2):
        v = in0.astype(np.float32) + in1
        u = v * v
        return v * (s0 + u * (s1 + imm2 * u))

    def _np_silu_fin(in0, in1, s0, s1, imm2):
        v = in0.astype(np.float32) + s0
        return v + v * np.clip(in1, -1.0, 1.0)

    v = Src0 + C3
    u = sq(v)
    spec1 = Spec(body=_spill_c3_to_src1(v * (C0 + u * (C1 + C2 * u))),
                 reference=_np_silu_z)
    vv = Src0 + C0
    spec2 = Spec(body=vv + vv * maxx(minn(Src1, One), Zero - One),
                 reference=_np_silu_fin)

    made = []
    for name, spec in (("SILU_Z_ANT", spec1), ("SILU_FIN_ANT", spec2)):
        opcode = dve_ops._CUSTOM_DVE_ROW_BASE + len(dve_ops.OPS)
        shas = {}
        for ver in ("v3", "v4"):
            try:
                uops = lower(spec, ver=ver)
                shas[ver] = DveOpSpec(
                    name=name, opcode=opcode, uops=uops, rd1_en=True
                ).sha(ver)
            except Exception:
                pass
        op = dve_ops.DveOp(name, spec, subdim=False, uops_sha=shas)
        dve_ops.OPS.append(op)
        dve_ops._SUB_OPCODE_FOR_NAME[name] = opcode
        dve_ops.CUSTOM_DVE_SPECS[name] = spec
        made.append(op)
    dve_ops._BY_NAME = {op.name: op for op in dve_ops.OPS}
    return tuple(made)


def _build_nc():
    import concourse.bass as bass
    import concourse.bacc as bacc
    import concourse.mybir as mybir
    import concourse.tile as tile

    f32 = mybir.dt.float32
    bf16 = mybir.dt.bfloat16

    silu_z_op, silu_fin_op = _register_dve_ops()

    nc = bacc.Bacc(
        "TRN2", target_bir_lowering=False, debug=False, num_devices=NCORES
    )
    bT_ext = nc.declare_dram_parameter("bT", [2, 128, N], f32, isOutput=False)
    abT_ext = nc.declare_dram_parameter("abT", [2, 128, NSLOT], f32, isOutput=False)
    wb2_ext = nc.declare_dram_parameter("wb2", [2, 128, 10], f32, isOutput=False)
    out_ext = nc.declare_dram_parameter("bondT", [5, TOT], bf16, isOutput=True)

    with tile.TileContext(nc) as tc:
        with (
            tc.tile_pool(name="weights", bufs=1) as wpool,
            tc.tile_pool(name="sil", bufs=3) as spool,
            tc.tile_pool(name="zt", bufs=2) as zpool,
            tc.tile_pool(name="psum", bufs=4, space=bass.MemorySpace.PSUM) as ppool,
        ):
            bT = wpool.tile([128, 2, N], f32)
            abT = wpool.tile([128, 2, NSLOT], f32)
            wb2f = wpool.tile([128, 2, 10], f32)
            wb2 = wpool.tile([128, 2, 10], bf16)
            big_ob = wpool.tile([5, TOT], bf16)
            for d in (0, 1):
                nc.sync.dma_start(bT[:, d, :], bT_ext[d])
                nc.sync.dma_start(abT[:, d, :], abT_ext[d])
                nc.sync.dma_start(wb2f[:, d, :], wb2_ext[d])
            nc.vector.tensor_copy(wb2[:], wb2f[:])

            # output DMA chunk boundaries, aligned to slot offsets
            slot_bounds = [0]
            for t in range(1, N_OUT_CHUNKS):
                target = TOT * t // N_OUT_CHUNKS
                slot_bounds.append(min(int(np.searchsorted(OFFS, target)), NSLOT))
            slot_bounds.append(NSLOT)

            for s in range(NSLOT):
                Fp = FP[s]
                off = int(OFFS[s])
                j0 = 8 * s + 1
                on_dve = s in DVE_SLOTS
                sil = spool.tile(
                    [128, 2, FP[0]], bf16, tag="sil_dve" if on_dve else "sil"
                )
                if on_dve:
                    # polynomial silu on DVE: sil = 2*silu(bT + bias); matmul
                    # later uses Wb2/2 for these slots
                    zt = zpool.tile([128, 2, FP[0]], f32, tag="zt")
                    for d in (0, 1):
                        nc.vector._custom_dve(
                            silu_z_op,
                            out=zt[:, d, :Fp],
                            in0=bT[:, d, j0 : j0 + Fp],
                            in1=abT[:, d, s : s + 1],
                            s0=PC0, s1=PC1, imm2=PC2,
                        )
                        nc.vector._custom_dve(
                            silu_fin_op,
                            out=sil[:, d, :Fp],
                            in0=bT[:, d, j0 : j0 + Fp],
                            in1=zt[:, d, :Fp],
                            s0=abT[:, d, s : s + 1],
                        )
                else:
                    for d in (0, 1):
                        nc.scalar.activation(
                            sil[:, d, :Fp],
                            bT[:, d, j0 : j0 + Fp],
                            mybir.ActivationFunctionType.Silu,
                            bias=abT[:, d, s : s + 1],
                        )
                w_lo, w_hi = (5, 10) if on_dve else (0, 5)
                c0 = 0
                while c0 < Fp:
                    ck = min(512, Fp - c0)
                    ps = ppool.tile([5, 512], f32, tag="ps")
                    for d in (0, 1):
                        nc.tensor.matmul(
                            ps[:, :ck],
                            wb2[:, d, w_lo:w_hi],
                            sil[:, d, c0 : c0 + ck],
                            start=(d == 0),
                            stop=(d == 1),
                        )
                    nc.vector.tensor_copy(
                        big_ob[:, off + c0 : off + c0 + ck], ps[:, :ck]
                    )
                    c0 += ck
                for t in range(1, N_OUT_CHUNKS + 1):
                    if slot_bounds[t] == s + 1 and slot_bounds[t] > slot_bounds[t - 1]:
                        a = int(OFFS[slot_bounds[t - 1]])
                        b = int(OFFS[slot_bounds[t]])
                        if b > a:
                            nc.sync.dma_start(out_ext[:, a:b], big_ob[:, a:b])
    nc.compile()
    return nc


def _get_nc():
    if "nc" not in _NC_CACHE:
        _NC_CACHE["nc"] = _build_nc()
    return _NC_CACHE["nc"]


def _silu(x):
    with np.errstate(over="ignore"):
        return (x / (1.0 + np.exp(-x))).astype(np.float32)


def _f32(x):
    return np.ascontiguousarray(np.asarray(x), dtype=np.float32)


def _kernel_impl(inputs, trace=False):
    from concourse.bass_utils import run_bass_kernel_spmd

    h = _f32(inputs["h"])
    Wa1, ba1 = _f32(inputs["Wa1"]), _f32(inputs["ba1"])
    Wa2, ba2 = _f32(inputs["Wa2"]), _f32(inputs["ba2"])
    Wb1, bb1 = _f32(inputs["Wb1"]), _f32(inputs["bb1"])
    Wb2, bb2 = _f32(inputs["Wb2"]), _f32(inputs["bb2"])
    Wv1, bv1 = _f32(inputs["Wv1"]), _f32(inputs["bv1"])
    Wv2, bv2 = _f32(inputs["Wv2"]), _f32(inputs["bv2"])

    # --- tiny host-side precompute (0.2% of total FLOPs) ---
    A = h @ Wb1[:D]            # [N, 256]
    B = h @ Wb1[D:]            # [N, 256]
    ABb = A + bb1[None, :]     # bias folded: silu(B[j] + ABb[i]) per pair
    BT = np.zeros((D, N + 7), dtype=np.float32)
    BT[:, :N] = B.T
    ABbT = ABb.T               # [256, N]

    in_maps = []
    for c in range(NCORES):
        slab = BT[:, c : c + N]                   # pre-shifted by core id
        cols = (8 * np.arange(NSLOT) + c).clip(max=N - 1)
        abT_c = ABbT[:, cols]                     # [256, 128]
        wb2_both = np.concatenate(
            [Wb2.reshape(2, 128, 5), (Wb2 * 0.5).reshape(2, 128, 5)], axis=2
        )
        in_maps.append(
            {
                "bT": np.ascontiguousarray(slab.reshape(2, 128, N)),
                "abT": np.ascontiguousarray(abT_c.reshape(2, 128, NSLOT)),
                "wb2": np.ascontiguousarray(wb2_both),
            }
        )

    nc = _get_nc()
    res = run_bass_kernel_spmd(nc, in_maps, core_ids=list(range(NCORES)), trace=trace)

    # --- assemble bond_logits [P, 5] ---
    bond = np.empty((P, 5), dtype=np.float32)
    for c in range(NCORES):
        slabT = np.asarray(res.results[c]["bondT"], dtype=np.float32)  # [5, TOT]
        for s in range(NSLOT):
            i = 8 * s + c
            if i > N - 2:
                continue
            F = N - 1 - i
            goff = i * (N - 1) - i * (i - 1) // 2
            q0 = int(OFFS[s])
            bond[goff : goff + F] = slabT[:, q0 : q0 + F].T
    bond += bb2[None, :]

    # --- tiny host-side MLPs (atom, valency) ---
    atom_logits = _silu(h @ Wa1 + ba1) @ Wa2 + ba2          # [N, 119]
    m = atom_logits.max(axis=-1, keepdims=True)
    e = np.exp(atom_logits - m)
    probs = (e / e.sum(axis=-1, keepdims=True)).astype(np.float32)
    vh = _silu(h @ Wv1[:D] + probs @ Wv1[D:] + bv1)
    valency = vh @ Wv2 + bv2                                 # [N, 1]

    ii, jj = np.triu_indices(N, k=1)
    atom_pairs = np.stack([ii, jj], axis=1).astype(np.int32)

    outs = (
        atom_logits.astype(np.float32),
        bond.astype(np.float32),
        valency.astype(np.float32),
        atom_pairs,
    )
    return outs, res


def kernel(**inputs):
    outs, _ = _kernel_impl(inputs, trace=False)
    return outs


# revision 29
# speedup vs baseline: 1.1281x; 1.1281x over previous
"""
Trainium2 Bass kernel for nn_AtomBondConsistencyLayer.

Math (see reference):
  atom_logits = silu(h@Wa1+ba1)@Wa2+ba2                         [N,119]
  bond_logits = silu(h[ii]@Wb1[:D] + h[jj]@Wb1[D:] + bb1)@Wb2+bb2  [P,5]
  valency    = silu(h@Wv1[:D] + softmax(atom_logits)@Wv1[D:]+bv1)@Wv2+bv2
  atom_pairs = triu pairs (i<j)                                  [P,2]

Key algebraic identity: h[ii]@W == (h@W)[ii], so the two huge [P,256]@[256,256]
GEMMs collapse to A=h@Wb1[:D], B=h@Wb1[D:] ([N,256] each, computed on host —
0.1% of FLOPs) plus the dominant per-pair work which runs on 8 NeuronCores:

  for each row-group i (pairs (i, j) with j=i+1..N-1):
      bond_hidden.T[:, j] = silu(B.T[:, j] + (A.T[:, i] + bb1))   <- ACT engine,
          bias folded into the activation's per-partition bias operand
      bond_logits.T = Wb2.T @ bond_hidden.T                        <- PE, bf16
  (+ bb2 added on host during assembly)

Sharding: group i -> core (i mod 8), so every core gets ~P/8 pairs AND ~128
groups (balanced instruction count). SPMD requires one identical graph on all
cores, so per-core inputs are pre-shifted slabs: slab_c[:, col] = B.T[:, col+c]
and slot widths padded to the core-0 width; junk columns are discarded on host.
"""

import sys
import numpy as np

sys.path.insert(0, "/opt/trn_rl_repo")

N = 1024
D = 256
NCORES = 8
NSLOT = 128  # slots per core; slot s on core c handles group i = 8*s + c
FP = [N - 1 - 8 * s for s in range(NSLOT)]  # padded slot widths (core-0 width)
OFFS = np.concatenate([[0], np.cumsum(FP)]).astype(np.int64)
TOT = int(OFFS[-1])  # 65920 padded pairs per core
P = N * (N - 1) // 2  # 523776

_NC_CACHE = {}

# matmul chunks of <=512 free columns, in slot order
CHUNKS = []
for _s in range(NSLOT):
    _c0 = 0
    while _c0 < FP[_s]:
        CHUNKS.append((_s, _c0, min(512, FP[_s] - _c0)))
        _c0 += 512
# slots offloaded to DVE via polynomial silu (~17% of elements), interleaved
# with ACT slots so both engines stream from the start
DVE_SLOTS = frozenset(range(0, 75, 5))

# big_ob output staging is folded into 4 row-groups at partition offsets
# 0/32/64/96 (engine APs need 32-aligned base partitions); tile is [101, GW]
NGRP = 4
GBOUND = [0]  # slot index boundaries of the groups
for _r in range(1, NGRP):
    _target = TOT * _r // NGRP
    GBOUND.append(min(int(np.searchsorted(OFFS, _target)), NSLOT))
GBOUND.append(NSLOT)
GROUP_OF = np.zeros(NSLOT, dtype=np.int64)
for _r in range(NGRP):
    GROUP_OF[GBOUND[_r] : GBOUND[_r + 1]] = _r
GOFF = [int(OFFS[GBOUND[_r]]) for _r in range(NGRP)]  # pair-col offset of group r
GW = max(int(OFFS[GBOUND[_r + 1]] - OFFS[GBOUND[_r]]) for _r in range(NGRP))
# degree-5 odd minimax-ish fit of tanh(x/2) on the data distribution:
# z = v*(PC0 + u*(PC1 + PC2*u)), u=v^2; 2*silu(v) ~= v + v*clamp(z,-1,1)
PC0, PC1, PC2 = 0.499859, -0.04079, 0.003025


def _register_dve_ops():
    import concourse.dve_ops as dve_ops
    from concourse.dve_spec import (
        Spec, Src0, Src1, C0, C1, C2, C3, Zero, One,
        sq, maxx, minn, lower, _spill_c3_to_src1,
    )
    from concourse.dve_uop import DveOpSpec

    if "SILU_Z_ANT" in dve_ops._SUB_OPCODE_FOR_NAME:
        return (dve_ops._BY_NAME["SILU_Z_ANT"], dve_ops._BY_NAME["SILU_FIN_ANT"])

    def _np_silu_z(in0, in1, s0, s1, imm2):
        v = in0.astype(np.float32) + in1
        u = v * v
        return v * (s0 + u * (s1 + imm2 * u))

    def _np_silu_fin(in0, in1, s0, s1, imm2):
        v = in0.astype(np.float32) + s0
        return v + v * np.clip(in1, -1.0, 1.0)

    v = Src0 + C3
    u = sq(v)
    spec1 = Spec(body=_spill_c3_to_src1(v * (C0 + u * (C1 + C2 * u))),
                 reference=_np_silu_z)
    vv = Src0 + C0
    spec2 = Spec(body=vv + vv * maxx(minn(Src1, One), Zero - One),
                 reference=_np_silu_fin)

    made = []
    for name, spec in (("SILU_Z_ANT", spec1), ("SILU_FIN_ANT", spec2)):
        opcode = dve_ops._CUSTOM_DVE_ROW_BASE + len(dve_ops.OPS)
        shas = {}
        for ver in ("v3", "v4"):
            try:
                uops = lower(spec, ver=ver)
                shas[ver] = DveOpSpec(
                    name=name, opcode=opcode, uops=uops, rd1_en=True
                ).sha(ver)
            except Exception:
                pass
        op = dve_ops.DveOp(name, spec, subdim=False, uops_sha=shas)
        dve_ops.OPS.append(op)
        dve_ops._SUB_OPCODE_FOR_NAME[name] = opcode
        dve_ops.CUSTOM_DVE_SPECS[name] = spec
        made.append(op)
    dve_ops._BY_NAME = {op.name: op for op in dve_ops.OPS}
    return tuple(made)


def _build_nc():
    import concourse.bass as bass
    import concourse.bacc as bacc
    import concourse.mybir as mybir
    import concourse.tile as tile

    f32 = mybir.dt.float32
    bf16 = mybir.dt.bfloat16

    silu_z_op, silu_fin_op = _register_dve_ops()

    nc = bacc.Bacc(
        "TRN2", target_bir_lowering=False, debug=False, num_devices=NCORES
    )
    bT_ext = nc.declare_dram_parameter("bT", [2, 128, N], f32, isOutput=False)
    abT_ext = nc.declare_dram_parameter("abT", [2, 128, NSLOT], f32, isOutput=False)
    wb2_ext = nc.declare_dram_parameter("wb2", [2, 128, 10], f32, isOutput=False)
    out_ext = nc.declare_dram_parameter("bondT", [101, GW], bf16, isOutput=True)

    with tile.TileContext(nc) as tc:
        with (
            tc.tile_pool(name="weights", bufs=1) as wpool,
            tc.tile_pool(name="sil", bufs=6) as spool,
            tc.tile_pool(name="zt", bufs=3) as zpool,
            tc.tile_pool(name="psum", bufs=6, space=bass.MemorySpace.PSUM) as ppool,
        ):
            bT = wpool.tile([128, 2, N], f32)
            abT = wpool.tile([128, 2, NSLOT], f32)
            wb2f = wpool.tile([128, 2, 10], f32)
            wb2 = wpool.tile([128, 2, 10], bf16)
            big_ob = wpool.tile([101, GW], bf16)
            for d in (0, 1):
                nc.sync.dma_start(bT[:, d, :], bT_ext[d])
                nc.sync.dma_start(abT[:, d, :], abT_ext[d])
                nc.sync.dma_start(wb2f[:, d, :], wb2_ext[d])
            nc.vector.tensor_copy(wb2[:], wb2f[:])

            # output DMA sub-chunks: ~4 per row-group, aligned to slot bounds
            slot_bounds = sorted(
                {
                    min(int(np.searchsorted(OFFS, OFFS[GBOUND[r]] + (OFFS[GBOUND[r + 1]] - OFFS[GBOUND[r]]) * q // 4)), GBOUND[r + 1])
                    for r in range(NGRP)
                    for q in range(1, 5)
                }
                | set(GBOUND)
            )

            for s in range(NSLOT):
                Fp = FP[s]
                off = int(OFFS[s])
                j0 = 8 * s + 1
                on_dve = s in DVE_SLOTS
                sil = spool.tile(
                    [128, 2, FP[0]], bf16, tag="sil_dve" if on_dve else "sil"
                )
                if on_dve:
                    # polynomial silu on DVE: sil = 2*silu(bT + bias); matmul
                    # later uses Wb2/2 for these slots
                    zt = zpool.tile([128, 2, FP[0]], f32, tag="zt")
                    for d in (0, 1):
                        nc.vector._custom_dve(
                            silu_z_op,
                            out=zt[:, d, :Fp],
                            in0=bT[:, d, j0 : j0 + Fp],
                            in1=abT[:, d, s : s + 1],
                            s0=PC0, s1=PC1, imm2=PC2,
                        )
                        nc.vector._custom_dve(
                            silu_fin_op,
                            out=sil[:, d, :Fp],
                            in0=bT[:, d, j0 : j0 + Fp],
                            in1=zt[:, d, :Fp],
                            s0=abT[:, d, s : s + 1],
                        )
                else:
                    for d in (0, 1):
                        nc.scalar.activation(
                            sil[:, d, :Fp],
                            bT[:, d, j0 : j0 + Fp],
                            mybir.ActivationFunctionType.Silu,
                            bias=abT[:, d, s : s + 1],
                        )
                w_lo, w_hi = (5, 10) if on_dve else (0, 5)
                r = int(GROUP_OF[s])
                base = off - GOFF[r]
                c0 = 0
                while c0 < Fp:
                    ck = min(512, Fp - c0)
                    ps = ppool.tile([5, 512], f32, tag="ps")
                    for d in (0, 1):
                        nc.tensor.matmul(
                            ps[:, :ck],
                            wb2[:, d, w_lo:w_hi],
                            sil[:, d, c0 : c0 + ck],
                            start=(d == 0),
                            stop=(d == 1),
                        )
                    nc.vector.tensor_copy(
                        big_ob[32 * r : 32 * r + 5, base + c0 : base + c0 + ck],
                        ps[:, :ck],
                    )
                    c0 += ck
                if (s + 1) in slot_bounds and (s + 1) > 0:
                    t = slot_bounds.index(s + 1)
                    if t > 0:
                        sa, sb = slot_bounds[t - 1], slot_bounds[t]
                        ra = int(GROUP_OF[sa])
                        a = int(OFFS[sa]) - GOFF[ra]
                        b = int(OFFS[sb]) - GOFF[ra]
                        if b > a:
                            nc.sync.dma_start(
                                out_ext[32 * ra : 32 * ra + 5, a:b],
                                big_ob[32 * ra : 32 * ra + 5, a:b],
                            )
    nc.compile()
    return nc


def _get_nc():
    if "nc" not in _NC_CACHE:
        _NC_CACHE["nc"] = _build_nc()
    return _NC_CACHE["nc"]


def _silu(x):
    with np.errstate(over="ignore"):
        return (x / (1.0 + np.exp(-x))).astype(np.float32)


def _f32(x):
    return np.ascontiguousarray(np.asarray(x), dtype=np.float32)


def _kernel_impl(inputs, trace=False):
    from concourse.bass_utils import run_bass_kernel_spmd

    h = _f32(inputs["h"])
    Wa1, ba1 = _f32(inputs["Wa1"]), _f32(inputs["ba1"])
    Wa2, ba2 = _f32(inputs["Wa2"]), _f32(inputs["ba2"])
    Wb1, bb1 = _f32(inputs["Wb1"]), _f32(inputs["bb1"])
    Wb2, bb2 = _f32(inputs["Wb2"]), _f32(inputs["bb2"])
    Wv1, bv1 = _f32(inputs["Wv1"]), _f32(inputs["bv1"])
    Wv2, bv2 = _f32(inputs["Wv2"]), _f32(inputs["bv2"])

    # --- tiny host-side precompute (0.2% of total FLOPs) ---
    A = h @ Wb1[:D]            # [N, 256]
    B = h @ Wb1[D:]            # [N, 256]
    ABb = A + bb1[None, :]     # bias folded: silu(B[j] + ABb[i]) per pair
    BT = np.zeros((D, N + 7), dtype=np.float32)
    BT[:, :N] = B.T
    ABbT = ABb.T               # [256, N]

    in_maps = []
    for c in range(NCORES):
        slab = BT[:, c : c + N]                   # pre-shifted by core id
        cols = (8 * np.arange(NSLOT) + c).clip(max=N - 1)
        abT_c = ABbT[:, cols]                     # [256, 128]
        wb2_both = np.concatenate(
            [Wb2.reshape(2, 128, 5), (Wb2 * 0.5).reshape(2, 128, 5)], axis=2
        )
        in_maps.append(
            {
                "bT": np.ascontiguousarray(slab.reshape(2, 128, N)),
                "abT": np.ascontiguousarray(abT_c.reshape(2, 128, NSLOT)),
                "wb2": np.ascontiguousarray(wb2_both),
            }
        )

    nc = _get_nc()
    res = run_bass_kernel_spmd(nc, in_maps, core_ids=list(range(NCORES)), trace=trace)

    # --- assemble bond_logits [P, 5] ---
    bond = np.empty((P, 5), dtype=np.float32)
    for c in range(NCORES):
        slabT = np.asarray(res.results[c]["bondT"], dtype=np.float32)  # [25, GW]
        for s in range(NSLOT):
            i = 8 * s + c
            if i > N - 2:
                continue
            F = N - 1 - i
            goff = i * (N - 1) - i * (i - 1) // 2
            r = int(GROUP_OF[s])
            base = int(OFFS[s]) - GOFF[r]
            bond[goff : goff + F] = slabT[32 * r : 32 * r + 5, base : base + F].T
    bond += bb2[None, :]

    # --- tiny host-side MLPs (atom, valency) ---
    atom_logits = _silu(h @ Wa1 + ba1) @ Wa2 + ba2          # [N, 119]
    m = atom_logits.max(axis=-1, keepdims=True)
    e = np.exp(atom_logits - m)
    probs = (e / e.sum(axis=-1, keepdims=True)).astype(np.float32)
    vh = _silu(h @ Wv1[:D] + probs @ Wv1[D:] + bv1)
    valency = vh @ Wv2 + bv2                                 # [N, 1]

    ii, jj = np.triu_indices(N, k=1)
    atom_pairs = np.stack([ii, jj], axis=1).astype(np.int32)

    outs = (
        atom_logits.astype(np.float32),
        bond.astype(np.float32),
        valency.astype(np.float32),
        atom_pairs,
    )
    return outs, res


def kernel(**inputs):
    outs, _ = _kernel_impl(inputs, trace=False)
    return outs


# revision 30
# speedup vs baseline: 1.3108x; 1.1620x over previous
"""
Trainium2 Bass kernel for nn_AtomBondConsistencyLayer.

Math (see reference):
  atom_logits = silu(h@Wa1+ba1)@Wa2+ba2                         [N,119]
  bond_logits = silu(h[ii]@Wb1[:D] + h[jj]@Wb1[D:] + bb1)@Wb2+bb2  [P,5]
  valency    = silu(h@Wv1[:D] + softmax(atom_logits)@Wv1[D:]+bv1)@Wv2+bv2
  atom_pairs = triu pairs (i<j)                                  [P,2]

Key algebraic identity: h[ii]@W == (h@W)[ii], so the two huge [P,256]@[256,256]
GEMMs collapse to A=h@Wb1[:D], B=h@Wb1[D:] ([N,256] each, computed on host —
0.1% of FLOPs) plus the dominant per-pair work which runs on 8 NeuronCores:

  for each row-group i (pairs (i, j) with j=i+1..N-1):
      bond_hidden.T[:, j] = silu(B.T[:, j] + (A.T[:, i] + bb1))   <- ACT engine,
          bias folded into the activation's per-partition bias operand
      bond_logits.T = Wb2.T @ bond_hidden.T                        <- PE, bf16
  (+ bb2 added on host during assembly)

Sharding: group i -> core (i mod 8), so every core gets ~P/8 pairs AND ~128
groups (balanced instruction count). SPMD requires one identical graph on all
cores, so per-core inputs are pre-shifted slabs: slab_c[:, col] = B.T[:, col+c]
and slot widths padded to the core-0 width; junk columns are discarded on host.
"""

import sys
import numpy as np

sys.path.insert(0, "/opt/trn_rl_repo")

N = 1024
D = 256
NCORES = 8
NSLOT = 128  # slots per core; slot s on core c handles group i = 8*s + c
FP = [N - 1 - 8 * s for s in range(NSLOT)]  # padded slot widths (core-0 width)
OFFS = np.concatenate([[0], np.cumsum(FP)]).astype(np.int64)
TOT = int(OFFS[-1])  # 65920 padded pairs per core
P = N * (N - 1) // 2  # 523776

_NC_CACHE = {}

# matmul chunks of <=512 free columns, in slot order
CHUNKS = []
for _s in range(NSLOT):
    _c0 = 0
    while _c0 < FP[_s]:
        CHUNKS.append((_s, _c0, min(512, FP[_s] - _c0)))
        _c0 += 512
# slots offloaded to DVE via polynomial silu (~17% of elements), interleaved
# with ACT slots so both engines stream from the start
DVE_SLOTS = frozenset(range(0, 75, 5))

# big_ob output staging is folded into 4 row-groups at partition offsets
# 0/32/64/96 (engine APs need 32-aligned base partitions); tile is [101, GW]
NGRP = 4
GBOUND = [0]  # slot index boundaries of the groups
for _r in range(1, NGRP):
    _target = TOT * _r // NGRP
    GBOUND.append(min(int(np.searchsorted(OFFS, _target)), NSLOT))
GBOUND.append(NSLOT)
GROUP_OF = np.zeros(NSLOT, dtype=np.int64)
for _r in range(NGRP):
    GROUP_OF[GBOUND[_r] : GBOUND[_r + 1]] = _r
GOFF = [int(OFFS[GBOUND[_r]]) for _r in range(NGRP)]  # pair-col offset of group r
GW = max(int(OFFS[GBOUND[_r + 1]] - OFFS[GBOUND[_r]]) for _r in range(NGRP))
# degree-5 odd minimax-ish fit of tanh(x/2) on the data distribution:
# z = v*(PC0 + u*(PC1 + PC2*u)), u=v^2; 2*silu(v) ~= v + v*clamp(z,-1,1)
PC0, PC1, PC2 = 0.499859, -0.04079, 0.003025


def _register_dve_ops():
    import concourse.dve_ops as dve_ops
    from concourse.dve_spec import (
        Spec, Src0, Src1, C0, C1, C2, C3, Zero, One,
        sq, maxx, minn, lower, _spill_c3_to_src1,
    )
    from concourse.dve_uop import DveOpSpec

    if "SILU_Z_ANT" in dve_ops._SUB_OPCODE_FOR_NAME:
        return (dve_ops._BY_NAME["SILU_Z_ANT"], dve_ops._BY_NAME["SILU_FIN_ANT"])

    def _np_silu_z(in0, in1, s0, s1, imm2):
        v = in0.astype(np.float32) + in1
        u = v * v
        return v * (s0 + u * (s1 + imm2 * u))

    def _np_silu_fin(in0, in1, s0, s1, imm2):
        v = in0.astype(np.float32) + s0
        return v + v * np.clip(in1, -1.0, 1.0)

    v = Src0 + C3
    u = sq(v)
    spec1 = Spec(body=_spill_c3_to_src1(v * (C0 + u * (C1 + C2 * u))),
                 reference=_np_silu_z)
    vv = Src0 + C0
    spec2 = Spec(body=vv + vv * maxx(minn(Src1, One), Zero - One),
                 reference=_np_silu_fin)

    made = []
    for name, spec in (("SILU_Z_ANT", spec1), ("SILU_FIN_ANT", spec2)):
        opcode = dve_ops._CUSTOM_DVE_ROW_BASE + len(dve_ops.OPS)
        shas = {}
        for ver in ("v3", "v4"):
            try:
                uops = lower(spec, ver=ver)
                shas[ver] = DveOpSpec(
                    name=name, opcode=opcode, uops=uops, rd1_en=True
                ).sha(ver)
            except Exception:
                pass
        op = dve_ops.DveOp(name, spec, subdim=False, uops_sha=shas)
        dve_ops.OPS.append(op)
        dve_ops._SUB_OPCODE_FOR_NAME[name] = opcode
        dve_ops.CUSTOM_DVE_SPECS[name] = spec
        made.append(op)
    dve_ops._BY_NAME = {op.name: op for op in dve_ops.OPS}
    return tuple(made)


def _build_nc():
    import concourse.bass as bass
    import concourse.bacc as bacc
    import concourse.mybir as mybir
    import concourse.tile as tile

    f32 = mybir.dt.float32
    bf16 = mybir.dt.bfloat16

    silu_z_op, silu_fin_op = _register_dve_ops()

    nc = bacc.Bacc(
        "TRN2", target_bir_lowering=False, debug=False, num_devices=NCORES
    )
    bT_ext = nc.declare_dram_parameter("bT", [2, 128, N], f32, isOutput=False)
    abT_ext = nc.declare_dram_parameter("abT", [2, 128, NSLOT], f32, isOutput=False)
    wb2_ext = nc.declare_dram_parameter("wb2", [2, 128, 10], f32, isOutput=False)
    out_ext = nc.declare_dram_parameter("bondT", [101, GW], bf16, isOutput=True)

    with tile.TileContext(nc) as tc:
        with (
            tc.tile_pool(name="weights", bufs=1) as wpool,
            tc.tile_pool(name="sil", bufs=6) as spool,
            tc.tile_pool(name="sild", bufs=3) as sdpool,
            tc.tile_pool(name="zt", bufs=2) as zpool,
            tc.tile_pool(name="psum", bufs=6, space=bass.MemorySpace.PSUM) as ppool,
        ):
            bT = wpool.tile([128, 2, N], f32)
            bTd = wpool.tile([128, 2, N], f32)  # DVE's own copy (avoids SBUF
            # read contention with ACT streaming the same region)
            abT = wpool.tile([128, 2, NSLOT], f32)
            wb2f = wpool.tile([128, 2, 10], f32)
            wb2 = wpool.tile([128, 2, 10], bf16)
            big_ob = wpool.tile([101, GW], bf16)
            for d in (0, 1):
                nc.sync.dma_start(bT[:, d, :], bT_ext[d])
                nc.sync.dma_start(bTd[:, d, :], bT_ext[d])
                nc.sync.dma_start(abT[:, d, :], abT_ext[d])
                nc.sync.dma_start(wb2f[:, d, :], wb2_ext[d])
            nc.vector.tensor_copy(wb2[:], wb2f[:])

            # output DMA sub-chunks: ~4 per row-group, aligned to slot bounds
            slot_bounds = sorted(
                {
                    min(int(np.searchsorted(OFFS, OFFS[GBOUND[r]] + (OFFS[GBOUND[r + 1]] - OFFS[GBOUND[r]]) * q // 4)), GBOUND[r + 1])
                    for r in range(NGRP)
                    for q in range(1, 5)
                }
                | set(GBOUND)
            )

            for s in range(NSLOT):
                Fp = FP[s]
                off = int(OFFS[s])
                j0 = 8 * s + 1
                on_dve = s in DVE_SLOTS
                pool = sdpool if on_dve else spool
                sil = pool.tile(
                    [128, 2, FP[0]], bf16, tag="sil_dve" if on_dve else "sil"
                )
                if on_dve:
                    # polynomial silu on DVE: sil = 2*silu(bT + bias); matmul
                    # later uses Wb2/2 for these slots
                    zt = zpool.tile([128, 2, FP[0]], f32, tag="zt")
                    for d in (0, 1):
                        nc.vector._custom_dve(
                            silu_z_op,
                            out=zt[:, d, :Fp],
                            in0=bTd[:, d, j0 : j0 + Fp],
                            in1=abT[:, d, s : s + 1],
                            s0=PC0, s1=PC1, imm2=PC2,
                        )
                        nc.vector._custom_dve(
                            silu_fin_op,
                            out=sil[:, d, :Fp],
                            in0=bTd[:, d, j0 : j0 + Fp],
                            in1=zt[:, d, :Fp],
                            s0=abT[:, d, s : s + 1],
                        )
                else:
                    for d in (0, 1):
                        nc.scalar.activation(
                            sil[:, d, :Fp],
                            bT[:, d, j0 : j0 + Fp],
                            mybir.ActivationFunctionType.Silu,
                            bias=abT[:, d, s : s + 1],
                        )
                w_lo, w_hi = (5, 10) if on_dve else (0, 5)
                r = int(GROUP_OF[s])
                base = off - GOFF[r]
                c0 = 0
                while c0 < Fp:
                    ck = min(512, Fp - c0)
                    ps = ppool.tile([5, 512], f32, tag="ps")
                    for d in (0, 1):
                        nc.tensor.matmul(
                            ps[:, :ck],
                            wb2[:, d, w_lo:w_hi],
                            sil[:, d, c0 : c0 + ck],
                            start=(d == 0),
                            stop=(d == 1),
                        )
                    nc.vector.tensor_copy(
                        big_ob[32 * r : 32 * r + 5, base + c0 : base + c0 + ck],
                        ps[:, :ck],
                    )
                    c0 += ck
                if (s + 1) in slot_bounds and (s + 1) > 0:
                    t = slot_bounds.index(s + 1)
                    if t > 0:
                        sa, sb = slot_bounds[t - 1], slot_bounds[t]
                        ra = int(GROUP_OF[sa])
                        a = int(OFFS[sa]) - GOFF[ra]
                        b = int(OFFS[sb]) - GOFF[ra]
                        if b > a:
                            nc.sync.dma_start(
                                out_ext[32 * ra : 32 * ra + 5, a:b],
                                big_ob[32 * ra : 32 * ra + 5, a:b],
                            )
    nc.compile()
    return nc


def _get_nc():
    if "nc" not in _NC_CACHE:
        _NC_CACHE["nc"] = _build_nc()
    return _NC_CACHE["nc"]


def _silu(x):
    with np.errstate(over="ignore"):
        return (x / (1.0 + np.exp(-x))).astype(np.float32)


def _f32(x):
    return np.ascontiguousarray(np.asarray(x), dtype=np.float32)


def _kernel_impl(inputs, trace=False):
    from concourse.bass_utils import run_bass_kernel_spmd

    h = _f32(inputs["h"])
    Wa1, ba1 = _f32(inputs["Wa1"]), _f32(inputs["ba1"])
    Wa2, ba2 = _f32(inputs["Wa2"]), _f32(inputs["ba2"])
    Wb1, bb1 = _f32(inputs["Wb1"]), _f32(inputs["bb1"])
    Wb2, bb2 = _f32(inputs["Wb2"]), _f32(inputs["bb2"])
    Wv1, bv1 = _f32(inputs["Wv1"]), _f32(inputs["bv1"])
    Wv2, bv2 = _f32(inputs["Wv2"]), _f32(inputs["bv2"])

    # --- tiny host-side precompute (0.2% of total FLOPs) ---
    A = h @ Wb1[:D]            # [N, 256]
    B = h @ Wb1[D:]            # [N, 256]
    ABb = A + bb1[None, :]     # bias folded: silu(B[j] + ABb[i]) per pair
    BT = np.zeros((D, N + 7), dtype=np.float32)
    BT[:, :N] = B.T
    ABbT = ABb.T               # [256, N]

    in_maps = []
    for c in range(NCORES):
        slab = BT[:, c : c + N]                   # pre-shifted by core id
        cols = (8 * np.arange(NSLOT) + c).clip(max=N - 1)
        abT_c = ABbT[:, cols]                     # [256, 128]
        wb2_both = np.concatenate(
            [Wb2.reshape(2, 128, 5), (Wb2 * 0.5).reshape(2, 128, 5)], axis=2
        )
        in_maps.append(
            {
                "bT": np.ascontiguousarray(slab.reshape(2, 128, N)),
                "abT": np.ascontiguousarray(abT_c.reshape(2, 128, NSLOT)),
                "wb2": np.ascontiguousarray(wb2_both),
            }
        )

    nc = _get_nc()
    res = run_bass_kernel_spmd(nc, in_maps, core_ids=list(range(NCORES)), trace=trace)

    # --- assemble bond_logits [P, 5] ---
    bond = np.empty((P, 5), dtype=np.float32)
    for c in range(NCORES):
        slabT = np.asarray(res.results[c]["bondT"], dtype=np.float32)  # [25, GW]
        for s in range(NSLOT):
            i = 8 * s + c
            if i > N - 2:
                continue
            F = N - 1 - i
            goff = i * (N - 1) - i * (i - 1) // 2
            r = int(GROUP_OF[s])
            base = int(OFFS[s]) - GOFF[r]
            bond[goff : goff + F] = slabT[32 * r : 32 * r + 5, base : base + F].T
    bond += bb2[None, :]

    # --- tiny host-side MLPs (atom, valency) ---
    atom_logits = _silu(h @ Wa1 + ba1) @ Wa2 + ba2          # [N, 119]
    m = atom_logits.max(axis=-1, keepdims=True)
    e = np.exp(atom_logits - m)
    probs = (e / e.sum(axis=-1, keepdims=True)).astype(np.float32)
    vh = _silu(h @ Wv1[:D] + probs @ Wv1[D:] + bv1)
    valency = vh @ Wv2 + bv2                                 # [N, 1]

    ii, jj = np.triu_indices(N, k=1)
    atom_pairs = np.stack([ii, jj], axis=1).astype(np.int32)

    outs = (
        atom_logits.astype(np.float32),
        bond.astype(np.float32),
        valency.astype(np.float32),
        atom_pairs,
    )
    return outs, res


def kernel(**inputs):
    outs, _ = _kernel_impl(inputs, trace=False)
    return outs
